# revision 18
# baseline (speedup 1.0000x reference)
"""Causal GQA self-attention (B=4, T=2048, C=2048, 16 Q heads / 8 KV heads,
hd=128) as a Bass/Tile SPMD kernel on 8 Trainium2 NeuronCores.

Sharding: core c = (batch b = c//2, head-group g = c%2). Each core handles one
batch and 8 Q heads / 4 KV heads. Wq/Wk/Wv column-sharded on the head dim, Wo
row-sharded; the host sums the two partial Wo products per batch.

Layouts (all transposed [feature, token] so every contraction is on the
partition dim): qT/kT = [d, t], v = [t, d], scores as S^T = [k, q],
output y^T = [o, t]. Matmuls in bf16 with fp32 PSUM accumulation.

Schedule: one continuous PE instruction stream per token block. ACT-gated
attention steps (score matmul -> exp -> masked diag -> DVE denominator
accumulate -> out matmul, with a 2-step software-pipeline lookahead) are
interleaved one-for-one with dense projection/Wo matmul "fillers" so the PE
never waits on the ScalarE exp latency and the HAM clock gate stays at 8/8.
All weights are SBUF-persistent, host-packed partition-major so every DMA
descriptor is a multi-KB contiguous run. The softmax denominator is
accumulated across k-blocks on the DVE in fp16 (2x tensor_tensor perf mode)
and reduced+broadcast across partitions with a single ones-matmul per head
(PE matmul cost depends only on the free dim, so this costs one 512-col
pass instead of one per k-block). The causal diagonal is trimmed at 128-col
granularity: fully-masked subtiles are never computed by score/exp/out.
A dummy-matmul warmup burst bridges the DMA prologue so the HAM clock gate
is already at 8/8 when real work starts.
"""

import sys
from collections import deque

import ml_dtypes
import numpy as np

sys.path.insert(0, "/opt/trn_rl_repo")

import concourse.bass as bass  # noqa: E402
import concourse.mybir as mybir  # noqa: E402
import concourse.tile as tile  # noqa: E402
from concourse import bacc, bass_isa  # noqa: E402
from concourse.bass_utils import run_bass_kernel_spmd  # noqa: E402

# Problem shape (hardcoded per contest contract).
B = 4
T = 2048
C = 2048
HD = 128
N_HEAD = 16
N_KV_HEAD = 8
NQH = N_HEAD // 2  # q heads per core
NKV = N_KV_HEAD // 2  # kv heads per core
TB = 512  # token block
NTB = T // TB
NCT = C // 128  # contraction tiles for the projections
NOG = C // 128  # output row tiles for Wo
SCALE = 1.0 / float(np.sqrt(HD))
LOOKAHEAD = 2

F32 = mybir.dt.float32
F16 = mybir.dt.float16
BF16 = mybir.dt.bfloat16
MULT = mybir.AluOpType.mult
ADD = mybir.AluOpType.add
EXP = mybir.ActivationFunctionType.Exp


def build_nc():
    nc = bacc.Bacc("TRN2", target_bir_lowering=False, debug=False, num_devices=8)

    # packed partition-major layouts (host pre-transposed for wide DMA runs):
    # xb[p, (tb*NCT+ct)*TB + t'] = x[b].T[ct*128+p, tb*TB+t']
    xb_d = nc.dram_tensor("xb_d", [128, NTB * NCT * TB], BF16, kind="ExternalInput")
    # wq[p, qh*(NCT*128) + ct*128 + j] = Wq.T[ct*128+p, qh*128+j]  (head-major)
    wq_d = nc.dram_tensor("wq_d", [128, NQH * NCT * 128], BF16, kind="ExternalInput")
    # wk/wv[p, ct*512 + j] = W.T[ct*128+p, j]  (ct-major)
    wk_d = nc.dram_tensor("wk_d", [128, NCT * NKV * HD], BF16, kind="ExternalInput")
    wv_d = nc.dram_tensor("wv_d", [128, NCT * NKV * HD], BF16, kind="ExternalInput")
    # wo[p, jh*C + j] = Wo_part.T[jh*128+p, j]  (contraction-tile-major)
    wo_d = nc.dram_tensor("wo_d", [128, NQH * C], BF16, kind="ExternalInput")
    cosdt = nc.dram_tensor("cosdt", [HD, T], BF16, kind="ExternalInput")
    nsindt = nc.dram_tensor("nsindt", [HD, T], BF16, kind="ExternalInput")
    tridt = nc.dram_tensor("tridt", [128, 128], BF16, kind="ExternalInput")
    onesdt = nc.dram_tensor("onesdt", [128, 128], F16, kind="ExternalInput")
        # packed output: yT_d[p, (og*NTB+tb)*TB + t'] = y_partial[og*128+p, tb*TB+t']
    yT = nc.dram_tensor("yT", [128, NOG * NTB * TB], BF16, kind="ExternalOutput")

    from contextlib import ExitStack

    with ExitStack() as es:
        tc = es.enter_context(tile.TileContext(nc))
        es.enter_context(nc.allow_low_precision("bf16 attention"))
        constp = es.enter_context(tc.tile_pool(name="const", bufs=1))
        perp = es.enter_context(tc.tile_pool(name="persist", bufs=1))
        strp = es.enter_context(tc.tile_pool(name="stream", bufs=2))
        xp = es.enter_context(tc.tile_pool(name="xp", bufs=2))
        qp = es.enter_context(tc.tile_pool(name="qt", bufs=8))
        expp = es.enter_context(tc.tile_pool(name="exs", bufs=4))
        exsump = es.enter_context(tc.tile_pool(name="exsum", bufs=2))
        sump = es.enter_context(tc.tile_pool(name="sums", bufs=1))
        outp = es.enter_context(tc.tile_pool(name="ot", bufs=16))
        tmpp = es.enter_context(tc.tile_pool(name="tmp", bufs=2))
        ysbp = es.enter_context(tc.tile_pool(name="ysb", bufs=2))
        projp = es.enter_context(tc.tile_pool(name="pp", bufs=3, space="PSUM"))
        spsum = es.enter_context(tc.tile_pool(name="sp", bufs=3, space="PSUM"))
        opsum = es.enter_context(tc.tile_pool(name="op", bufs=2, space="PSUM"))

        # ---- persistent tiles ----
        tri = constp.tile([128, 128], BF16, tag="tri", name="tri")
        ones128 = constp.tile([128, 128], F16, tag="ones128", name="ones128")
        wk_all = perp.tile([128, NCT * NKV * HD], BF16, tag="wk", name="wk_all")
        wv_all = perp.tile([128, NCT * NKV * HD], BF16, tag="wv", name="wv_all")
        wq_all = perp.tile([128, NQH * NCT * 128], BF16, tag="wq", name="wq_all")
        wo_all = perp.tile([128, NQH * C], BF16, tag="wo", name="wo_all")
        dummy = constp.tile([128, 128], BF16, tag="dummy", name="dummy")
        kT = [perp.tile([HD, T], BF16, tag=f"kT{h}", name=f"kTt{h}") for h in range(NKV)]
        vT = [perp.tile([128, NKV * HD], BF16, tag=f"v{i}", name=f"vt{i}") for i in range(T // 128)]

        # ---- prologue DMAs (ordered so K-proj of block 0 can start ASAP) ----
        nc.sync.dma_start(tri[:], tridt[:])
        nc.sync.dma_start(ones128[:], onesdt[:])

        # ---- PE warm-up: dummy matmuls bridge the DMA prologue so the HAM
        # activity window stays busy and real matmuls start at the 2.4 GHz
        # clock instead of the cold 1.2 GHz default. Gated on a GpSimd memset
        # (GpSimd boots ~4 us before the DMA data engines); sized to end
        # right when the first projection operands land.
        nc.gpsimd.memset(dummy[:], 0.0)
        wps = projp.tile([128, 128], F32, tag="pp", name="warmps")
        for _ in range(56):
            nc.tensor.matmul(wps[:], dummy[:], dummy[:], start=True, stop=True)
        cos_t = {}
        nsin_t = {}
        cb = strp.tile([HD, TB], BF16, tag="cosb", name="cosb0")
        nc.sync.dma_start(cb[:], cosdt[:, 0:TB])
        cos_t[0] = cb
        nb = strp.tile([HD, TB], BF16, tag="nsinb", name="nsinb0")
        nc.sync.dma_start(nb[:], nsindt[:, 0:TB])
        nsin_t[0] = nb
        xb_set = {}  # tb -> big tile [128, NCT*TB]
        xb_set[0] = xp.tile([128, NCT * TB], BF16, tag="xb", name="xb0")
        WKC = NCT * NKV * HD // 4
        XC = NCT * TB // 4
        for c4 in range(4):  # interleave wk / x(0) chunks
            nc.sync.dma_start(
                wk_all[:, c4 * WKC : (c4 + 1) * WKC], wk_d[:, c4 * WKC : (c4 + 1) * WKC]
            )
            nc.sync.dma_start(
                xb_set[0][:, c4 * XC : (c4 + 1) * XC], xb_d[:, c4 * XC : (c4 + 1) * XC]
            )
        for c4 in range(4):
            nc.sync.dma_start(
                wv_all[:, c4 * WKC : (c4 + 1) * WKC], wv_d[:, c4 * WKC : (c4 + 1) * WKC]
            )
        WQH = NCT * 128
        for qh in range(NQH):  # head-major so Q-proj streams incrementally
            nc.sync.dma_start(
                wq_all[:, qh * WQH : (qh + 1) * WQH], wq_d[:, qh * WQH : (qh + 1) * WQH]
            )
        for c4 in range(4):
            WOC = NQH * C // 4
            nc.sync.dma_start(
                wo_all[:, c4 * WOC : (c4 + 1) * WOC], wo_d[:, c4 * WOC : (c4 + 1) * WOC]
            )

        # ---- shared emission helpers ----
        qts_t = {}  # (tb, h) -> tile
        outs_t = {}  # (tb, h) -> tile
        emitted = set()

        def rope(dst, src_psum, tb):
            """dst = src*cos + rot_half(src)*sin, [d, t] layout; nsin table is
            pre-rotated by 64 partitions with sign folded in, so both halves
            are plain multiplies with aligned input base partitions."""
            cosb, nsinb = cos_t[tb], nsin_t[tb]
            t0 = tmpp.tile([HD, TB], BF16, tag="t0", name="ropet0")
            nc.scalar.copy(t0[:], src_psum[:])
            nc.vector.tensor_mul(dst, t0[:], cosb[:])
            t2 = tmpp.tile([HD, TB], BF16, tag="t2", name="ropet2")
            nc.vector.tensor_mul(t2[0:64, :], t0[64:128, :], nsinb[64:128, :])
            nc.vector.tensor_mul(t2[64:128, :], t0[0:64, :], nsinb[0:64, :])
            nc.vector.tensor_add(dst, dst, t2[:])

        def build_fillers(tb):
            """Dense PE work for segment tb: K/V/Q projections of block tb,
            Wo of block tb-1, plus DMA prefetches for block tb+1. Each entry
            is (emit_fn, tag_or_None)."""
            fillers = []
            tsl = slice(tb * TB, (tb + 1) * TB)

            # K projection + RoPE -> kT[kv][:, tsl]
            xb = xb_set[tb]
            for kv in range(NKV):
                kps = projp.tile([128, TB], F32, tag="pp", name=f"kps{tb}_{kv}")
                for ct in range(NCT):
                    def mk(kps=kps, kv=kv, ct=ct, xb=xb):
                        nc.tensor.matmul(
                            kps[:],
                            wk_all[:, ct * 512 + kv * 128 : ct * 512 + (kv + 1) * 128],
                            xb[:, ct * TB : (ct + 1) * TB],
                            start=(ct == 0),
                            stop=(ct == NCT - 1),
                        )
                    fillers.append((mk, None))
                def mkr(kps=kps, kv=kv, tb=tb, tsl=tsl):
                    rope(kT[kv][:, tsl], kps, tb)
                fillers.append((mkr, ("k", tb, kv)))

            # V projection ([t, d] layout) -> vT[4*tb + i]
            for i in range(4):
                vps = projp.tile([128, NKV * HD], F32, tag="pp", name=f"vps{tb}_{i}")
                for ct in range(NCT):
                    def mv(vps=vps, i=i, ct=ct, xb=xb):
                        nc.tensor.matmul(
                            vps[:],
                            xb[:, ct * TB + i * 128 : ct * TB + (i + 1) * 128],
                            wv_all[:, ct * 512 : (ct + 1) * 512],
                            start=(ct == 0),
                            stop=(ct == NCT - 1),
                        )
                    fillers.append((mv, None))
                def mvc(vps=vps, i=i, tb=tb):
                    nc.vector.tensor_copy(vT[4 * tb + i][:], vps[:])
                fillers.append((mvc, ("v", tb, i)))

            # Q projection + RoPE -> qts, with x/cos prefetch DMAs sprinkled in
            for qh in range(NQH):
                qps = projp.tile([128, TB], F32, tag="pp", name=f"qps{tb}_{qh}")
                for ct in range(NCT):
                    def mq(qps=qps, qh=qh, ct=ct, xb=xb):
                        nc.tensor.matmul(
                            qps[:],
                            wq_all[:, qh * 2048 + ct * 128 : qh * 2048 + (ct + 1) * 128],
                            xb[:, ct * TB : (ct + 1) * TB],
                            start=(ct == 0),
                            stop=(ct == NCT - 1),
                        )
                    fillers.append((mq, None))
                def mqr(qps=qps, qh=qh, tb=tb):
                    qt = qp.tile([HD, TB], BF16, tag="qt", name=f"qt{tb}_{qh}")
                    rope(qt[:], qps, tb)
                    qts_t[(tb, qh)] = qt
                fillers.append((mqr, ("q", tb, qh)))
                if tb + 1 < NTB and qh < 4:
                    def mdx(c4=qh, tb=tb):
                        if c4 == 0:
                            xb_set[tb + 1] = xp.tile(
                                [128, NCT * TB], BF16, tag="xb", name=f"xb{tb+1}"
                            )
                        XSEG = NCT * TB
                        XC = XSEG // 4
                        nc.sync.dma_start(
                            xb_set[tb + 1][:, c4 * XC : (c4 + 1) * XC],
                            xb_d[:, (tb + 1) * XSEG + c4 * XC : (tb + 1) * XSEG + (c4 + 1) * XC],
                        )
                    fillers.append((mdx, None))
                if tb + 1 < NTB and qh == 4:
                    ntsl = slice((tb + 1) * TB, (tb + 2) * TB)
                    def mdc(tb=tb, ntsl=ntsl):
                        cb = strp.tile([HD, TB], BF16, tag="cosb", name=f"cosb{tb+1}")
                        nc.sync.dma_start(cb[:], cosdt[:, ntsl])
                        cos_t[tb + 1] = cb
                        nb = strp.tile([HD, TB], BF16, tag="nsinb", name=f"nsinb{tb+1}")
                        nc.sync.dma_start(nb[:], nsindt[:, ntsl])
                        nsin_t[tb + 1] = nb
                    fillers.append((mdc, None))

            # Wo of block tb-1
            if tb > 0:
                fillers.extend(build_wo_fillers(tb - 1))
            return fillers

        def build_wo_fillers(wtb):
            fillers = []
            tsl = slice(wtb * TB, (wtb + 1) * TB)
            for og in range(NOG):
                yps = projp.tile([128, TB], F32, tag="pp", name=f"yps{wtb}_{og}")
                for jh in range(NQH):
                    def mw(yps=yps, og=og, jh=jh, wtb=wtb):
                        nc.tensor.matmul(
                            yps[:],
                            wo_all[:, jh * C + og * 128 : jh * C + (og + 1) * 128],
                            outs_t[(wtb, jh)][:],
                            start=(jh == 0),
                            stop=(jh == NQH - 1),
                        )
                    fillers.append((mw, None))
                def mwc(yps=yps, og=og, wtb=wtb):
                    ysb = ysbp.tile([128, TB], BF16, tag="ysb", name="ysb")
                    nc.vector.tensor_copy(ysb[:], yps[:])
                    oc = (og * NTB + wtb) * TB
                    nc.sync.dma_start(yT[:, oc : oc + TB], ysb[:])
                fillers.append((mwc, None))
            return fillers

        def build_steps(tb):
            """Attention steps for block tb: S^T = k x q per (head, k-block),
            causally trimmed at 128-col granularity."""
            steps = []
            ktmax = 4 * tb + 4
            for h in range(NQH):
                for kt in range(ktmax):
                    m = kt - 4 * tb
                    lo = 128 * m if m > 0 else 0
                    needs = [("q", tb, h), ("k", kt // 4, h // 2), ("v", kt // 4, kt % 4)]
                    steps.append(
                        dict(
                            tb=tb, h=h, kt=kt, m=m, lo=lo,
                            first=(kt == 0), last=(kt == ktmax - 1),
                            needs=needs, ex=None,
                        )
                    )
            return steps

        head_state = {}  # h -> (ops_, exsum)

        def emit_score_phase(s):
            tb, h, kt, lo = s["tb"], s["h"], s["kt"], s["lo"]
            hv = h // 2
            if s["first"]:
                ops_ = opsum.tile([HD, TB], F32, tag="op", name=f"aop{tb}_{h}")
                exsum = exsump.tile([128, TB], F16, tag="exsum", name="exsum")
                head_state[(tb, h)] = (ops_, exsum)
            _, exsum = head_state[(tb, h)]
            sps = spsum.tile([128, TB], F32, tag="sp", name="sps")
            nc.tensor.matmul(
                sps[:, lo:TB],
                kT[hv][:, kt * 128 : (kt + 1) * 128],
                qts_t[(tb, h)][:, lo:TB],
                start=True,
                stop=True,
            )
            ex = expp.tile([128, TB], BF16, tag="ex", name="ex")
            nc.scalar.activation(ex[:, lo:TB], sps[:, lo:TB], EXP, scale=SCALE)
            if s["m"] >= 0:
                dcs = slice(128 * s["m"], 128 * (s["m"] + 1))
                nc.vector.tensor_mul(ex[:, dcs], ex[:, dcs], tri[:])
            if s["first"]:
                nc.vector.tensor_copy(exsum[:], ex[:])
            else:
                nc.vector.tensor_add(exsum[:, lo:TB], ex[:, lo:TB], exsum[:, lo:TB])
            s["ex"] = ex

        def emit_out_phase(s):
            tb, h, kt, lo = s["tb"], s["h"], s["kt"], s["lo"]
            hv = h // 2
            ops_, exsum = head_state[(tb, h)]
            nc.tensor.matmul(
                ops_[:, lo:TB],
                vT[kt][:, hv * 128 : (hv + 1) * 128],
                s["ex"][:, lo:TB],
                start=s["first"],
                stop=s["last"],
            )
            if s["last"]:
                # softmax denominator: reduce over k partitions AND broadcast
                # to all 128 partitions in one PE matmul against a ones matrix
                den = spsum.tile([128, TB], F32, tag="sp", name="den")
                nc.tensor.matmul(den[:], ones128[:], exsum[:], start=True, stop=True)
                sums = sump.tile([128, TB], F32, tag="sums", name="sums")
                nc.vector.reciprocal_approx_fast(sums[:], den[:])
                ot = outp.tile([HD, TB], BF16, tag="ot", name=f"ot{tb}_{h}")
                nc.vector.tensor_mul(ot[:], ops_[:], sums[:])
                outs_t[(tb, h)] = ot

        def run_segment(steps, fillers):
            fc = 0

            def emit_filler():
                nonlocal fc
                fn, tag = fillers[fc]
                fn()
                if tag is not None:
                    emitted.add(tag)
                fc += 1

            pend = deque()
            for s in steps:
                while fc < len(fillers) and not all(t in emitted for t in s["needs"]):
                    emit_filler()
                emit_score_phase(s)
                pend.append(s)
                if len(pend) > LOOKAHEAD:
                    emit_out_phase(pend.popleft())
                if fc < len(fillers):
                    emit_filler()
            while pend:
                emit_out_phase(pend.popleft())
            while fc < len(fillers):
                emit_filler()

        for tb in range(NTB):
            run_segment(build_steps(tb), build_fillers(tb))
        run_segment([], build_wo_fillers(NTB - 1))

    nc.compile()
    return nc


def _host_consts():
    inv_freq = 1.0 / (10000.0 ** (np.arange(0, HD, 2, dtype=np.float32) / HD))
    t = np.arange(T, dtype=np.float32)
    freqs = np.outer(t, inv_freq)  # [T, HD/2]
    freqs = np.repeat(freqs, 2, axis=-1)  # [T, HD]
    cos = np.cos(freqs).astype(np.float32).T.copy()  # [HD, T]
    sin = np.sin(freqs).astype(np.float32).T.copy()
    # rotated-by-64 signed sin table: row d holds the multiplier that pairs
    # with x[(d+64)%128]; rows 64..127 carry -sin[0:64], rows 0..63 +sin[64:128]
    nsin = np.empty_like(sin)
    nsin[0:64, :] = sin[64:128, :]
    nsin[64:128, :] = -sin[0:64, :]

    bf = ml_dtypes.bfloat16
    kp = np.arange(128)[:, None]
    qf = np.arange(128)[None, :]
    tri = (kp <= qf).astype(bf)

    return {
        "cosdt": np.ascontiguousarray(cos.astype(bf)),
        "nsindt": np.ascontiguousarray(nsin.astype(bf)),
        "tridt": tri,
        "onesdt": np.ones((128, 128), dtype=np.float16),
    }


_NC_CACHE = None


def _get_nc():
    global _NC_CACHE
    if _NC_CACHE is None:
        _NC_CACHE = build_nc()
    return _NC_CACHE


def kernel(x, Wq, Wk, Wv, Wo, _trace=False):
    x = np.asarray(x, dtype=np.float32)
    Wq = np.asarray(Wq, dtype=np.float32)
    Wk = np.asarray(Wk, dtype=np.float32)
    Wv = np.asarray(Wv, dtype=np.float32)
    Wo = np.asarray(Wo, dtype=np.float32)

    nc = _get_nc()
    consts = _host_consts()

    bf = ml_dtypes.bfloat16
    # packed partition-major host layouts (see dram tensor comments)
    xbs = []
    for b in range(B):
        xT_b = x[b].T.astype(bf)  # [C, T]
        xbs.append(np.ascontiguousarray(
            xT_b.reshape(NCT, 128, NTB, TB).transpose(1, 2, 0, 3).reshape(128, NTB * NCT * TB)
        ))
    wqs, wks, wvs, wos = [], [], [], []
    for g in range(2):
        wqT_g = Wq[1024 * g : 1024 * (g + 1), :].T.astype(bf)  # [C, 1024]
        wqs.append(np.ascontiguousarray(
            wqT_g.reshape(NCT, 128, NQH, 128).transpose(1, 2, 0, 3).reshape(128, NQH * NCT * 128)
        ))
        wkT_g = Wk[512 * g : 512 * (g + 1), :].T.astype(bf)  # [C, 512]
        wks.append(np.ascontiguousarray(
            wkT_g.reshape(NCT, 128, NKV * HD).transpose(1, 0, 2).reshape(128, NCT * NKV * HD)
        ))
        wvT_g = Wv[512 * g : 512 * (g + 1), :].T.astype(bf)
        wvs.append(np.ascontiguousarray(
            wvT_g.reshape(NCT, 128, NKV * HD).transpose(1, 0, 2).reshape(128, NCT * NKV * HD)
        ))
        woT_g = Wo[:, 1024 * g : 1024 * (g + 1)].T.astype(bf)  # [1024, C]
        wos.append(np.ascontiguousarray(
            woT_g.reshape(NQH, 128, C).transpose(1, 0, 2).reshape(128, NQH * C)
        ))

    in_maps = []
    for c in range(8):
        b, g = c // 2, c % 2
        im = {
            "xb_d": xbs[b],
            "wq_d": wqs[g],
            "wk_d": wks[g],
            "wv_d": wvs[g],
            "wo_d": wos[g],
        }
        im.update(consts)
        in_maps.append(im)

    res = run_bass_kernel_spmd(nc, in_maps, core_ids=list(range(8)), trace=_trace)

    y = np.empty((B, T, C), dtype=np.float32)
    for b in range(B):
        ya = np.asarray(res.results[2 * b]["yT"]).astype(np.float32)
        yb = np.asarray(res.results[2 * b + 1]["yT"]).astype(np.float32)
        yp = (ya + yb).reshape(128, NOG, NTB, TB).transpose(1, 0, 2, 3).reshape(C, T)
        y[b] = yp.T
    if _trace:
        return y, res
    return y
